# revision 1
# baseline (speedup 1.0000x reference)
"""HGT layer (heterogeneous graph transformer) on 8 trn2 NeuronCores.

Strategy (dst-sharded, fully on-device message passing):
  * Edges of each relation are sorted by dst on host and sharded across the 8
    cores by dst range (core c owns dst rows [c*3750, (c+1)*3750) of the
    relevant node type). No collectives are needed: node features h0/h1 are
    replicated (inputs), per-edge K/V projections are computed on device from
    gathered h rows, and Q is a small per-core table (own dst rows only).
  * Per relation, edges are packed into "blocks": <=128 consecutive dsts and
    <=640 edges (5 chunks of 128). Per chunk we:
      - dma_gather (transposed) the 128 source h rows (bf16)  -> lhsT
      - 2 matmuls against [Wk_eff | Wv_eff] -> kv PSUM [128e, 512]
      - dma_gather (row-major) the 128 q rows from the q table
      - score s = per-head sum(q * k) (DVE mul + reduce), ex = exp(s) (ACT)
      - rhs = [v * ex_broadcast | ex] bf16
      - banded segment-sum: matmul(U += S^T_chunk @ rhs) accumulating in PSUM
    After 5 chunks: t = U[:, :256] / (U[:, 256:264] + eps) per head.
  * Softmax max-subtraction is skipped (scores ~ N(0,1); exp is safe; the
    result is mathematically identical). The dst-constant score bias term
    (q . bk_eff) cancels in the per-dst softmax, so bk is dropped exactly.
    bv_eff is folded in after normalization; bq is added into the q table.
  * n1 receives rel0 and rel2 with shared block boundaries; t1 = (t0+t2)/2.
  * Output: t is PE-transposed, matmul'd with Wa, and combined with the
    pre-scaled skip rows (host-packed h*(1-alpha) + alpha*ba).
Outputs are written in packed-block order; host unpacks to [2, 30000, 256].
"""

import math
import os

import numpy as np
import ml_dtypes

import concourse.bass as bass
import concourse.bacc as bacc
import concourse.tile as tile
from concourse import mybir
from concourse.bass_utils import run_bass_kernel_spmd
from concourse.masks import make_identity

BF16 = ml_dtypes.bfloat16

N = 30000
D = 256
H = 8
DK = 32
E = 160000
NCORES = 8
RPC = N // NCORES          # dst rows per core
BLK = 640                  # max edges per block
CPB = BLK // 128           # chunks per block (5)
GRP = 1                    # blocks per gather group
GIDX = GRP * BLK           # indices per gather (1280)
ICOLS = BLK // 16          # idx columns per block (40)
QTR = 3840                 # q table rows (3750 padded to 128 mult)

SRC_OF_REL = (0, 1, 1)     # node type of src per relation
DST_OF_REL = (1, 0, 1)     # node type of dst per relation

_cache = {}


# ----------------------------------------------------------------------------
# Host preprocessing
# ----------------------------------------------------------------------------

def _block_diag(mats):
    # mats: [H, DK, DK] -> [D, D] block diagonal
    out = np.zeros((H * mats.shape[1], H * mats.shape[2]), np.float32)
    for h in range(mats.shape[0]):
        out[h * DK:(h + 1) * DK, h * DK:(h + 1) * DK] = mats[h]
    return out


def _wrap_idx(idx_groups):
    """idx_groups: list of int arrays each of length GIDX. Returns
    [128, len*GIDX//16] int16 in the 16-partition-wrapped dma_gather layout."""
    ncol = len(idx_groups) * (GIDX // 16)
    out = np.zeros((128, ncol), np.int16)
    for g, arr in enumerate(idx_groups):
        w = np.asarray(arr, np.int16).reshape(GIDX // 16, 16).T  # [16, 80]
        out[:, g * (GIDX // 16):(g + 1) * (GIDX // 16)] = np.tile(w, (8, 1))
    return out


def _pack_blocks(seg_counts_list):
    """Greedy-pack consecutive dsts into blocks.
    seg_counts_list: list of per-dst edge counts arrays (all same length RPC);
    a block must satisfy <=128 dsts and <=BLK edges in EVERY relation given.
    Returns list of (d_lo, nd)."""
    n = len(seg_counts_list[0])
    blocks = []
    i = 0
    while i < n:
        d0 = i
        e = [0] * len(seg_counts_list)
        while i < n and (i - d0) < 128:
            ok = all(e[k] + seg_counts_list[k][i] <= BLK
                     for k in range(len(seg_counts_list)))
            if not ok:
                break
            for k in range(len(seg_counts_list)):
                e[k] += seg_counts_list[k][i]
            i += 1
        assert i > d0, "single dst segment exceeds BLK edges"
        blocks.append((d0, i - d0))
    return blocks


def prep(inputs):
    h0 = np.asarray(inputs['h0'], np.float32)
    h1 = np.asarray(inputs['h1'], np.float32)
    Wk = np.asarray(inputs['Wk'], np.float32)
    bk = np.asarray(inputs['bk'], np.float32)
    Wq = np.asarray(inputs['Wq'], np.float32)
    bq = np.asarray(inputs['bq'], np.float32)
    Wv = np.asarray(inputs['Wv'], np.float32)
    bv = np.asarray(inputs['bv'], np.float32)
    Wa = np.asarray(inputs['Wa'], np.float32)
    ba = np.asarray(inputs['ba'], np.float32)
    rel_att = np.asarray(inputs['rel_att'], np.float32)
    rel_msg = np.asarray(inputs['rel_msg'], np.float32)
    rel_pri = np.asarray(inputs['rel_pri'], np.float32)
    skip = np.asarray(inputs['skip'], np.float32)

    alpha = 1.0 / (1.0 + np.exp(-skip))          # [2]
    hs = [h0, h1]

    # effective projections (att/msg/pri folded); bk dropped (cancels in the
    # per-dst softmax: its score contribution is constant within a segment).
    Wk_eff, Wv_eff, bv_eff = [], [], []
    for r in range(3):
        st = SRC_OF_REL[r]
        A = _block_diag(rel_att[r])
        M = _block_diag(rel_msg[r])
        scale = np.repeat(rel_pri[r] / math.sqrt(DK), DK)  # [256]
        Wk_eff.append((Wk[st] @ A) * scale[None, :])
        Wv_eff.append(Wv[st] @ M)
        bv_eff.append(bv[st] @ M)

    # edge sorting by dst
    edges = []
    for r, (skey, dkey) in enumerate((('src0', 'dst0'), ('src1', 'dst1'),
                                      ('src2', 'dst2'))):
        src = np.asarray(inputs[skey], np.int64)
        dst = np.asarray(inputs[dkey], np.int64)
        order = np.argsort(dst, kind='stable')
        ssrc = src[order]
        sdst = dst[order]
        counts = np.bincount(dst, minlength=N)
        starts = np.zeros(N + 1, np.int64)
        np.cumsum(counts, out=starts[1:])
        edges.append((ssrc, sdst, counts, starts))

    # per-core packing
    per_core = []
    for c in range(NCORES):
        lo = c * RPC
        cnt1 = edges[1][2][lo:lo + RPC]
        blocks0 = _pack_blocks([cnt1])                       # n0 side (rel1)
        cnt0 = edges[0][2][lo:lo + RPC]
        cnt2 = edges[2][2][lo:lo + RPC]
        blocks1 = _pack_blocks([cnt0, cnt2])                 # n1 side (rel0+2)
        per_core.append((blocks0, blocks1))

    NB0 = max(len(pc[0]) for pc in per_core)
    NB1 = max(len(pc[1]) for pc in per_core)
    NB0 += NB0 % 2
    NB1 += NB1 % 2

    # bf16 replicated tables
    h0b = h0.astype(BF16)
    h1b = h1.astype(BF16)

    # weights, chunked for matmul rhs
    wq_t = np.stack([Wq[t].reshape(2, 128, D) for t in range(2)]).astype(BF16)
    wkv_t = np.stack([
        np.concatenate([Wk_eff[r], Wv_eff[r]], axis=1).reshape(2, 128, 2 * D)
        for r in range(3)]).astype(BF16)
    wa_t = np.stack([Wa[t].reshape(2, 128, D) for t in range(2)]).astype(BF16)
    bq_t = bq.copy()                                          # [2, 256] f32
    bv_t = np.stack([bv_eff[1], 0.5 * (bv_eff[0] + bv_eff[2])])  # [2,256]
    use_bv = bool(np.abs(bv_t).max() > 0)

    in_maps = []
    unpack = []
    for c in range(NCORES):
        lo = c * RPC
        blocks0, blocks1 = per_core[c]
        m = {
            'h0b': h0b, 'h1b': h1b,
            'wq': wq_t, 'wkv': wkv_t, 'wa': wa_t,
            'bq2': bq_t, 'bvt': bv_t.astype(np.float32),
        }
        # q-projection gather indices (own rows, transposed gather)
        qp = np.zeros((2, 128, QTR // 16), np.int16)
        for t in range(2):
            rows = np.minimum(lo + np.arange(QTR), lo + RPC - 1)
            qp[t] = _wrap_idx([rows[g * GIDX:(g + 1) * GIDX]
                               for g in range(QTR // GIDX)])
        m['qpidx'] = qp

        # per relation edge data
        rel_blocks = {0: blocks1, 1: blocks0, 2: blocks1}
        rel_nb = {0: NB1, 1: NB0, 2: NB1}
        for r in range(3):
            ssrc, sdst, counts, starts = edges[r]
            blocks = rel_blocks[r]
            nb = rel_nb[r]
            sidx = np.zeros((nb * BLK,), np.int64)
            qidx = np.zeros((nb * BLK,), np.int64)
            st = np.zeros((nb, 128, CPB * 128), BF16)
            for b, (d_lo, nd) in enumerate(blocks):
                e0 = starts[lo + d_lo]
                e1 = starts[lo + d_lo + nd]
                ne = e1 - e0
                assert ne <= BLK
                if ne == 0:
                    continue
                sidx[b * BLK: b * BLK + ne] = ssrc[e0:e1]
                qidx[b * BLK: b * BLK + ne] = sdst[e0:e1] - lo
                slot = (sdst[e0:e1] - (lo + d_lo)).astype(np.int64)
                j = np.arange(ne)
                st[b, j % 128, (j // 128) * 128 + slot] = 1.0
            groups = [np.concatenate([sidx[g * GIDX:(g + 1) * GIDX]])
                      for g in range(nb // GRP)]
            m[f'sidx{r}'] = _wrap_idx(groups)
            groups = [qidx[g * GIDX:(g + 1) * GIDX]
                      for g in range(nb // GRP)]
            m[f'qidx{r}'] = _wrap_idx(groups)
            m[f'st{r}'] = st

        # skip rows, packed; pre-scaled: h*(1-a) + a*ba
        hsk = np.zeros(((NB0 + NB1) * 128, D), np.float32)
        for i, (d_lo, nd) in enumerate(blocks0):
            hsk[i * 128: i * 128 + nd] = (hs[0][lo + d_lo: lo + d_lo + nd]
                                          * (1 - alpha[0]) + alpha[0] * ba[0])
        for i, (d_lo, nd) in enumerate(blocks1):
            hsk[(NB0 + i) * 128:(NB0 + i) * 128 + nd] = (
                hs[1][lo + d_lo: lo + d_lo + nd] * (1 - alpha[1])
                + alpha[1] * ba[1])
        m['hsk'] = hsk
        in_maps.append(m)
        unpack.append((blocks0, blocks1))

    meta = dict(NB0=NB0, NB1=NB1, alpha=(float(alpha[0]), float(alpha[1])),
                use_bv=use_bv)
    return in_maps, unpack, meta


# ----------------------------------------------------------------------------
# Device program
# ----------------------------------------------------------------------------

def build_program(NB0, NB1, alpha, use_bv):
    fp32 = mybir.dt.float32
    bf16 = mybir.dt.bfloat16
    i16 = mybir.dt.int16
    nc = bacc.Bacc('TRN2', target_bir_lowering=False, debug=False,
                   num_devices=NCORES,
                   dynamic_dma_scratch_size=65536)

    h0b = nc.dram_tensor('h0b', [N, D], bf16, kind='ExternalInput')
    h1b = nc.dram_tensor('h1b', [N, D], bf16, kind='ExternalInput')
    wq = nc.dram_tensor('wq', [2, 2, 128, D], bf16, kind='ExternalInput')
    wkv = nc.dram_tensor('wkv', [3, 2, 128, 2 * D], bf16, kind='ExternalInput')
    wa = nc.dram_tensor('wa', [2, 2, 128, D], bf16, kind='ExternalInput')
    bq2 = nc.dram_tensor('bq2', [2, D], fp32, kind='ExternalInput')
    bvt = nc.dram_tensor('bvt', [2, D], fp32, kind='ExternalInput')
    qpidx = nc.dram_tensor('qpidx', [2, 128, QTR // 16], i16,
                           kind='ExternalInput')
    sidx = [nc.dram_tensor(f'sidx{r}', [128, (NB1 if r != 1 else NB0) * ICOLS],
                           i16, kind='ExternalInput') for r in range(3)]
    qidx = [nc.dram_tensor(f'qidx{r}', [128, (NB1 if r != 1 else NB0) * ICOLS],
                           i16, kind='ExternalInput') for r in range(3)]
    st = [nc.dram_tensor(f'st{r}', [(NB1 if r != 1 else NB0), 128, CPB * 128],
                         bf16, kind='ExternalInput') for r in range(3)]
    hsk = nc.dram_tensor('hsk', [(NB0 + NB1) * 128, D], fp32,
                         kind='ExternalInput')
    outp = nc.dram_tensor('outp', [(NB0 + NB1) * 128, D], fp32,
                          kind='ExternalOutput')
    qtab = [nc.dram_tensor(f'q{t}tab', [QTR, D], bf16, kind='Internal')
            for t in range(2)]

    htab = [h0b, h1b]
    REL_NB = {0: NB1, 1: NB0, 2: NB1}

    with tile.TileContext(nc) as tc:
        with (
            tc.tile_pool(name='singles', bufs=1) as singles,
            tc.tile_pool(name='gpool', bufs=4) as gpool,
            tc.tile_pool(name='spool', bufs=4) as spool,
            tc.tile_pool(name='work', bufs=8) as work,
            tc.tile_pool(name='opool', bufs=3) as opool,
            tc.tile_pool(name='psA', bufs=2, space='PSUM') as psA,
            tc.tile_pool(name='psU', bufs=2, space='PSUM') as psU,
            tc.tile_pool(name='psT', bufs=2, space='PSUM') as psT,
            tc.tile_pool(name='psO', bufs=2, space='PSUM') as psO,
        ):
            from concourse import library_config
            nc.gpsimd.load_library(library_config.mlp)
            gidx_reg = nc.gpsimd.to_reg(GIDX)

            ident = singles.tile([128, 128], bf16)
            make_identity(nc, ident[:])

            # resident weights
            wq_sb = singles.tile([128, 2, 2, D], bf16)
            nc.sync.dma_start(out=wq_sb[:],
                              in_=wq[:].rearrange('a b p n -> p a b n'))
            wkv_sb = singles.tile([128, 3, 2, 2 * D], bf16)
            nc.sync.dma_start(out=wkv_sb[:],
                              in_=wkv[:].rearrange('a b p n -> p a b n'))
            wa_sb = singles.tile([128, 2, 2, D], bf16)
            nc.sync.dma_start(out=wa_sb[:],
                              in_=wa[:].rearrange('a b p n -> p a b n'))
            bq_sb = singles.tile([128, 2, D], fp32)
            bv_sb = singles.tile([128, 2, D], fp32)
            for t in range(2):
                src = bq2[t:t + 1, :]
                nc.sync.dma_start(out=bq_sb[:, t, :],
                                  in_=src.to_broadcast([128, D]))
                src = bvt[t:t + 1, :]
                nc.sync.dma_start(out=bv_sb[:, t, :],
                                  in_=src.to_broadcast([128, D]))

            # resident indices
            qp_sb = singles.tile([128, 2, QTR // 16], i16)
            nc.sync.dma_start(out=qp_sb[:],
                              in_=qpidx[:].rearrange('a p n -> p a n'))
            sidx_sb = []
            qidx_sb = []
            for r in range(3):
                t1 = singles.tile([128, REL_NB[r] * ICOLS], i16,
                                  tag=f'sidx{r}')
                nc.sync.dma_start(out=t1[:], in_=sidx[r][:])
                sidx_sb.append(t1)
                t2 = singles.tile([128, REL_NB[r] * ICOLS], i16,
                                  tag=f'qidx{r}')
                nc.sync.dma_start(out=t2[:], in_=qidx[r][:])
                qidx_sb.append(t2)

            # t-store for rel0 results (n1 side)
            tstore = singles.tile([128, NB1, D], bf16)

            # ---------------- phase: q tables ----------------
            for t in range(2):
                for g in range(QTR // GIDX):
                    gth = gpool.tile([128, 2, GIDX], bf16, tag='gq')
                    nc.gpsimd.dma_gather(
                        out_ap=gth[:],
                        in_ap=htab[t][:],
                        idxs_ap=qp_sb[:, t,
                                      g * (GIDX // 16):(g + 1) * (GIDX // 16)],
                        num_idxs=GIDX, num_idxs_reg=gidx_reg,
                        elem_size=D, transpose=True)
                    for j in range(GIDX // 128):
                        qp_ps = psA.tile([128, 2 * D], fp32, tag='kv')
                        for ci in range(2):
                            nc.tensor.matmul(
                                out=qp_ps[:, :D],
                                lhsT=gth[:, ci, j * 128:(j + 1) * 128],
                                rhs=wq_sb[:, t, ci, :],
                                start=(ci == 0), stop=(ci == 1))
                        qs = work.tile([128, D], bf16, tag='qs')
                        nc.vector.tensor_add(qs[:], qp_ps[:, :D],
                                             bq_sb[:, t, :])
                        row = g * GIDX + j * 128
                        nc.sync.dma_start(out=qtab[t][row:row + 128, :],
                                          in_=qs[:])

            tc.strict_bb_all_engine_barrier()

            # ---------------- relation passes ----------------
            def rel_pass(r, mode):
                # mode: 'out' (rel1), 'store' (rel0), 'combine' (rel2)
                nb = REL_NB[r]
                side = DST_OF_REL[r]
                half = 0.5 if r != 1 else 1.0
                out_off = 0 if side == 0 else NB0
                for g in range(nb // GRP):
                    sg = gpool.tile([128, 2, GIDX], bf16, tag='gs')
                    nc.gpsimd.dma_gather(
                        out_ap=sg[:], in_ap=htab[SRC_OF_REL[r]][:],
                        idxs_ap=sidx_sb[r][:, g * (GIDX // 16):
                                           (g + 1) * (GIDX // 16)],
                        num_idxs=GIDX, num_idxs_reg=gidx_reg,
                        elem_size=D, transpose=True)
                    qg = gpool.tile([128, GIDX // 128, D], bf16, tag='gqg')
                    nc.gpsimd.dma_gather(
                        out_ap=qg[:], in_ap=qtab[side][:],
                        idxs_ap=qidx_sb[r][:, g * (GIDX // 16):
                                           (g + 1) * (GIDX // 16)],
                        num_idxs=GIDX, num_idxs_reg=gidx_reg,
                        elem_size=D, transpose=False)
                    for bb in range(GRP):
                        b = g * GRP + bb
                        stt = spool.tile([128, CPB * 128], bf16, tag='st')
                        nc.sync.dma_start(out=stt[:], in_=st[r][b])
                        u = psU.tile([128, 264], fp32, tag='u')
                        for j in range(CPB):
                            ci = bb * CPB + j
                            kv = psA.tile([128, 2 * D], fp32, tag='kv')
                            for cc in range(2):
                                nc.tensor.matmul(
                                    out=kv[:],
                                    lhsT=sg[:, cc, ci * 128:(ci + 1) * 128],
                                    rhs=wkv_sb[:, r, cc, :],
                                    start=(cc == 0), stop=(cc == 1))
                            p = work.tile([128, D], fp32, tag='p')
                            nc.vector.tensor_mul(p[:], qg[:, ci, :],
                                                 kv[:, :D])
                            s8 = work.tile([128, H], fp32, tag='s8')
                            nc.vector.reduce_sum(
                                s8[:], p[:].rearrange('p (h d) -> p h d',
                                                      d=DK),
                                axis=mybir.AxisListType.X)
                            ex = work.tile([128, H], fp32, tag='ex')
                            nc.scalar.activation(
                                ex[:], s8[:], mybir.ActivationFunctionType.Exp)
                            rhs = work.tile([128, 264], bf16, tag='rhs')
                            exb = ex[:]
                            exb = bass.AP(tensor=exb.tensor, offset=exb.offset,
                                          ap=[*exb.ap, [0, DK]])
                            nc.vector.tensor_mul(
                                rhs[:, :D].rearrange('p (h d) -> p h d', d=DK),
                                kv[:, D:].rearrange('p (h d) -> p h d', d=DK),
                                exb)
                            nc.scalar.activation(
                                rhs[:, D:D + H], ex[:],
                                mybir.ActivationFunctionType.Copy)
                            nc.tensor.matmul(
                                out=u[:], lhsT=stt[:, j * 128:(j + 1) * 128],
                                rhs=rhs[:], start=(j == 0), stop=(j == CPB - 1))
                        # normalize
                        rcp = work.tile([128, H], fp32, tag='rcp')
                        nc.vector.tensor_scalar_add(rcp[:], u[:, D:D + H],
                                                    1e-20)
                        nc.vector.reciprocal(rcp[:], rcp[:])
                        if half != 1.0:
                            nc.vector.tensor_scalar_mul(rcp[:], rcp[:], half)
                        rcpb = rcp[:]
                        rcpb = bass.AP(tensor=rcpb.tensor, offset=rcpb.offset,
                                       ap=[*rcpb.ap, [0, DK]])
                        if mode == 'store':
                            nc.vector.tensor_mul(
                                tstore[:, b, :].rearrange(
                                    'p (h d) -> p h d', d=DK),
                                u[:, :D].rearrange('p (h d) -> p h d', d=DK),
                                rcpb)
                            continue
                        t_sb = opool.tile([128, D], bf16, tag='t')
                        nc.vector.tensor_mul(
                            t_sb[:].rearrange('p (h d) -> p h d', d=DK),
                            u[:, :D].rearrange('p (h d) -> p h d', d=DK),
                            rcpb)
                        if mode == 'combine':
                            nc.vector.tensor_add(t_sb[:], t_sb[:],
                                                 tstore[:, b, :])
                        if use_bv:
                            nc.vector.tensor_add(t_sb[:], t_sb[:],
                                                 bv_sb[:, side, :])
                        # output: transpose, matmul Wa, skip-combine
                        tts = opool.tile([128, 2, 128], bf16, tag='tts')
                        for cc in range(2):
                            tp = psT.tile([128, 128], bf16, tag='tp')
                            nc.tensor.transpose(
                                tp[:], t_sb[:, cc * 128:(cc + 1) * 128],
                                ident[:])
                            nc.vector.tensor_copy(tts[:, cc, :], tp[:])
                        om = psO.tile([128, D], fp32, tag='om')
                        for cc in range(2):
                            nc.tensor.matmul(
                                out=om[:], lhsT=tts[:, cc, :],
                                rhs=wa_sb[:, side, cc, :],
                                start=(cc == 0), stop=(cc == 1))
                        hs_t = opool.tile([128, D], fp32, tag='hs')
                        row = (out_off + b) * 128
                        nc.sync.dma_start(out=hs_t[:],
                                          in_=hsk[row:row + 128, :])
                        o_sb = opool.tile([128, D], fp32, tag='o')
                        nc.vector.scalar_tensor_tensor(
                            out=o_sb[:], in0=om[:], scalar=alpha[side],
                            in1=hs_t[:], op0=mybir.AluOpType.mult,
                            op1=mybir.AluOpType.add)
                        nc.sync.dma_start(out=outp[row:row + 128, :],
                                          in_=o_sb[:])

            rel_pass(1, 'out')
            rel_pass(0, 'store')
            rel_pass(2, 'combine')

    nc.compile()
    return nc


# ----------------------------------------------------------------------------
# Entry point
# ----------------------------------------------------------------------------

def _run(inputs, trace=False):
    in_maps, unpack, meta = prep(inputs)
    key = (meta['NB0'], meta['NB1'], meta['alpha'], meta['use_bv'])
    if key not in _cache:
        _cache[key] = build_program(meta['NB0'], meta['NB1'], meta['alpha'],
                                    meta['use_bv'])
    nc = _cache[key]
    res = run_bass_kernel_spmd(nc, in_maps, core_ids=list(range(NCORES)),
                               trace=trace)
    NB0 = meta['NB0']
    out = np.zeros((2, N, D), np.float32)
    for c in range(NCORES):
        lo = c * RPC
        op = res.results[c]['outp']
        blocks0, blocks1 = unpack[c]
        for i, (d_lo, nd) in enumerate(blocks0):
            out[0, lo + d_lo: lo + d_lo + nd] = op[i * 128: i * 128 + nd]
        for i, (d_lo, nd) in enumerate(blocks1):
            out[1, lo + d_lo: lo + d_lo + nd] = op[(NB0 + i) * 128:
                                                   (NB0 + i) * 128 + nd]
    return out, res


def kernel(**inputs):
    out, _ = _run(inputs, trace=False)
    return out



# revision 27
# speedup vs baseline: 1.5975x; 1.5975x over previous
"""HGT layer (heterogeneous graph transformer) on 8 trn2 NeuronCores.

Strategy (dst-sharded, window-aligned, fully on-device message passing):
  * Edges of each relation are sorted by dst on host and sharded across the
    8 cores by dst range (core c owns local dst rows [0, 3750) of node rows
    [c*3750, (c+1)*3750)). No collectives: node features h0/h1 (bf16) are
    replicated inputs; K/V are projected on device per edge from gathered
    h rows; Q lives in SBUF (computed on device from the core's own rows).
  * Dst rows are processed in 128-aligned WINDOWS (30 per side). Window w
    of relation r is processed in chunks_w[r][w] chunks of 128 edge slots,
    where chunks_w = max over cores (so the SPMD program is uniform; the
    per-core variation lives in the data). Per chunk:
      - lhsT slice of a transposed dma_gather of src h rows (2048/gather)
      - 2 matmuls against [Wk_eff | Wv_eff]      -> kv PSUM [128e, 512]
      - qe = matmul(lhsT=S_chunk, rhs=Q_window)  -> per-edge q, no gather
      - p = k * qe (DVE, bf16), s = per-head reduce, ex = exp(s) (ACT,
        written as bf16 directly into rhs[:, 256:264])
      - rhs[:, :256] = v * ex_broadcast (DVE)
      - banded segment-sum: matmul(U += S_chunk^T @ rhs) accum in PSUM
    After the window's chunks: t = U[:, :256] / (U[:, 256:264] + eps).
  * S_chunk is a 0/1 [slot, edge] matrix with slot = local_dst % 128 --
    the same matrix provides both the segment-sum and the slot->edge Q
    broadcast. Softmax max-subtraction is skipped (scores ~ N(0,1)); the
    dst-constant bk term cancels in the per-dst softmax and is dropped.
  * n1 receives rel0 and rel2 (shared windows); t1 = (t0+t2)/2. Output:
    t is PE-transposed, matmul'd with Wa, combined with pre-scaled skip
    rows (host-packed h*(1-alpha) + alpha*ba). Output rows are window-
    aligned == contiguous local dst rows; host unpack is a slice copy.
  * The output phase of window w is emitted while window w+1's chunks are
    in flight (1-window software pipeline) to keep the PE queue moving.
"""

import math
import os

import numpy as np
import ml_dtypes

import concourse.bass as bass
import concourse.bacc as bacc
import concourse.tile as tile
from concourse import mybir
from concourse.bass_utils import run_bass_kernel_spmd
from concourse.masks import make_identity

BF16 = ml_dtypes.bfloat16

N = 30000
D = 256
H = 8
DK = 32
E = 160000
NCORES = 8
RPC = N // NCORES          # dst rows per core (3750)
W = (RPC + 127) // 128     # dst windows per side (30)
QTR = W * 128              # padded q rows (3840)
NPAD = 30720               # padded h table rows (so lo+QTR stays in range)
GIDX = int(os.environ.get('K_GIDX', '640'))    # indices per gather
                                               # (>640 crashes the exec unit)
GCH = GIDX // 128          # chunks per gather group
K_SKIP_GATHER = bool(int(os.environ.get('K_SKIP_GATHER', '0')))
K_SKIP_REL = bool(int(os.environ.get('K_SKIP_REL', '0')))
K_SKIP_FLUSH = bool(int(os.environ.get('K_SKIP_FLUSH', '0')))

SRC_OF_REL = (0, 1, 1)     # node type of src per relation
DST_OF_REL = (1, 0, 1)     # node type of dst per relation

_cache = {}


# ----------------------------------------------------------------------------
# Host preprocessing
# ----------------------------------------------------------------------------

def _block_diag(mats):
    # mats: [H, DK, DK] -> [D, D] block diagonal
    out = np.zeros((H * mats.shape[1], H * mats.shape[2]), np.float32)
    for h in range(mats.shape[0]):
        out[h * DK:(h + 1) * DK, h * DK:(h + 1) * DK] = mats[h]
    return out


def _wrap_idx(flat):
    """flat: int array, length multiple of GIDX. Returns [128, len//16] int16
    in the 16-partition-wrapped dma_gather layout (replicated to 128)."""
    ng = len(flat) // GIDX
    out = np.zeros((128, ng * (GIDX // 16)), np.int16)
    for g in range(ng):
        w = np.asarray(flat[g * GIDX:(g + 1) * GIDX],
                       np.int16).reshape(GIDX // 16, 16).T  # [16, 128]
        out[:, g * (GIDX // 16):(g + 1) * (GIDX // 16)] = np.tile(w, (8, 1))
    return out


def prep(inputs):
    h0 = np.asarray(inputs['h0'], np.float32)
    h1 = np.asarray(inputs['h1'], np.float32)
    Wk = np.asarray(inputs['Wk'], np.float32)
    bk = np.asarray(inputs['bk'], np.float32)  # dropped (cancels in softmax)
    Wq = np.asarray(inputs['Wq'], np.float32)
    bq = np.asarray(inputs['bq'], np.float32)
    Wv = np.asarray(inputs['Wv'], np.float32)
    bv = np.asarray(inputs['bv'], np.float32)
    Wa = np.asarray(inputs['Wa'], np.float32)
    ba = np.asarray(inputs['ba'], np.float32)
    rel_att = np.asarray(inputs['rel_att'], np.float32)
    rel_msg = np.asarray(inputs['rel_msg'], np.float32)
    rel_pri = np.asarray(inputs['rel_pri'], np.float32)
    skip = np.asarray(inputs['skip'], np.float32)

    alpha = 1.0 / (1.0 + np.exp(-skip))          # [2]
    hs = [h0, h1]

    # effective projections (att/msg/pri folded)
    Wk_eff, Wv_eff, bv_eff = [], [], []
    for r in range(3):
        st = SRC_OF_REL[r]
        A = _block_diag(rel_att[r])
        M = _block_diag(rel_msg[r])
        scale = np.repeat(rel_pri[r] / math.sqrt(DK), DK)  # [256]
        Wk_eff.append((Wk[st] @ A) * scale[None, :])
        Wv_eff.append(Wv[st] @ M)
        bv_eff.append(bv[st] @ M)

    # padded bf16 replicated node tables
    h0b = np.zeros((NPAD, D), BF16)
    h0b[:N] = h0.astype(BF16)
    h1b = np.zeros((NPAD, D), BF16)
    h1b[:N] = h1.astype(BF16)

    # weights, chunked for matmul rhs
    wq_t = np.stack([Wq[t].reshape(2, 128, D) for t in range(2)]).astype(BF16)
    wkv_t = np.stack([
        np.concatenate([Wk_eff[r], Wv_eff[r]], axis=1).reshape(2, 128, 2 * D)
        for r in range(3)]).astype(BF16)
    wa_t = np.stack([Wa[t].reshape(2, 128, D) for t in range(2)]).astype(BF16)
    bq_t = bq.copy()                                          # [2, 256]
    bv_t = np.stack([bv_eff[1], 0.5 * (bv_eff[0] + bv_eff[2])])  # [2,256]
    use_bq = bool(np.abs(bq_t).max() > 0)
    use_bv = bool(np.abs(bv_t).max() > 0)

    # --- per-relation edge schedule (uniform across cores) ---------------
    # rel_edges[r][c] = (ssrc_local, sdst_local) sorted by dst, local ids
    rel_edges = []
    rel_wcnt = []       # [r][c] -> per-window counts [W]
    for r, (skey, dkey) in enumerate((('src0', 'dst0'), ('src1', 'dst1'),
                                      ('src2', 'dst2'))):
        src = np.asarray(inputs[skey], np.int64)
        dst = np.asarray(inputs[dkey], np.int64)
        order = np.argsort(dst, kind='stable')
        ssrc = src[order]
        sdst = dst[order]
        counts = np.bincount(dst, minlength=N)
        starts = np.zeros(N + 1, np.int64)
        np.cumsum(counts, out=starts[1:])
        per_core = []
        per_wcnt = []
        for c in range(NCORES):
            lo = c * RPC
            e0, e1 = starts[lo], starts[lo + RPC]
            loc = sdst[e0:e1] - lo
            per_core.append((ssrc[e0:e1], loc))
            per_wcnt.append(np.bincount(loc // 128, minlength=W))
        rel_edges.append(per_core)
        rel_wcnt.append(per_wcnt)

    # uniform chunk schedule: chunks_w[r][w] = max over cores, >= 1
    chunks_w = []
    for r in range(3):
        cw = np.maximum(1, -(-np.stack(rel_wcnt[r]).max(axis=0) // 128))
        chunks_w.append(cw.astype(np.int64))
    total_chunks = [int(cw.sum()) for cw in chunks_w]
    ngroups = [-(-tc // GCH) for tc in total_chunks]
    cw0 = [np.concatenate([[0], np.cumsum(cw)]) for cw in chunks_w]

    in_maps = []
    for c in range(NCORES):
        lo = c * RPC
        m = {'h0b': h0b, 'h1b': h1b,
             'wq': wq_t, 'wkv': wkv_t, 'wa': wa_t}
        if use_bq:
            m['bq2'] = bq_t
        if use_bv:
            m['bvt'] = bv_t.astype(np.float32)

        for r in range(3):
            ssrc, loc = rel_edges[r][c]
            wcnt = rel_wcnt[r][c]
            wstart = np.zeros(W + 1, np.int64)
            np.cumsum(wcnt, out=wstart[1:])
            # per-edge slot in its window's chunk grid
            we = loc // 128
            k = np.arange(len(loc)) - wstart[we]       # pos within window
            ci = cw0[r][we] + k // 128                 # global chunk id
            col = ci * 128 + (k % 128)
            sidx = np.zeros(ngroups[r] * GIDX, np.int64)
            sidx[col] = ssrc
            # S in both orientations: st = [slot, chunk-edge] (lhsT of the
            # qe broadcast matmul), sx = [chunk-edge-pos, slot] (lhsT of the
            # U segment-sum matmul).
            st = np.zeros((128, total_chunks[r] * 128), BF16)
            st[loc % 128, col] = 1.0
            sx = np.zeros((128, total_chunks[r] * 128), BF16)
            sx[k % 128, ci * 128 + loc % 128] = 1.0
            m[f'sidx{r}'] = _wrap_idx(sidx)
            m[f'st{r}'] = st
            m[f'sx{r}'] = sx

        # skip rows, window-aligned == contiguous local rows; pre-scaled
        hsk = np.zeros((2 * QTR, D), np.float32)
        rows = np.minimum(lo + np.arange(QTR), N - 1)
        for t in range(2):
            hsk[t * QTR:(t + 1) * QTR] = (hs[t][rows] * (1 - alpha[t])
                                          + alpha[t] * ba[t])
        m['hsk'] = hsk
        # own rows transposed for the on-device Q projection:
        # hownT[t, cc, p, n] = h_t[lo + n, cc*128 + p]
        hownT = np.zeros((2, 2, 128, QTR), BF16)
        for t in range(2):
            ht = hs[t][rows].astype(BF16).T          # [256, QTR]
            hownT[t] = ht.reshape(2, 128, QTR)
        m['hownT'] = hownT
        in_maps.append(m)

    meta = dict(alpha=(float(alpha[0]), float(alpha[1])),
                use_bq=use_bq, use_bv=use_bv,
                chunks_w=tuple(tuple(int(x) for x in cw) for cw in chunks_w),
                ngroups=tuple(ngroups))
    return in_maps, meta


# ----------------------------------------------------------------------------
# Device program
# ----------------------------------------------------------------------------

def build_program(meta):
    fp32 = mybir.dt.float32
    bf16 = mybir.dt.bfloat16
    i16 = mybir.dt.int16
    alpha = meta['alpha']
    use_bq = meta['use_bq']
    use_bv = meta['use_bv']
    chunks_w = meta['chunks_w']
    ngroups = meta['ngroups']
    total_chunks = [sum(cw) for cw in chunks_w]

    nc = bacc.Bacc('TRN2', target_bir_lowering=False, debug=False,
                   num_devices=NCORES,
                   dynamic_dma_scratch_size=65536)

    h0b = nc.dram_tensor('h0b', [NPAD, D], bf16, kind='ExternalInput')
    h1b = nc.dram_tensor('h1b', [NPAD, D], bf16, kind='ExternalInput')
    wq = nc.dram_tensor('wq', [2, 2, 128, D], bf16, kind='ExternalInput')
    wkv = nc.dram_tensor('wkv', [3, 2, 128, 2 * D], bf16, kind='ExternalInput')
    wa = nc.dram_tensor('wa', [2, 2, 128, D], bf16, kind='ExternalInput')
    if use_bq:
        bq2 = nc.dram_tensor('bq2', [2, D], fp32, kind='ExternalInput')
    if use_bv:
        bvt = nc.dram_tensor('bvt', [2, D], fp32, kind='ExternalInput')
    hownT = nc.dram_tensor('hownT', [2, 2, 128, QTR], bf16,
                           kind='ExternalInput')
    sidx = [nc.dram_tensor(f'sidx{r}', [128, ngroups[r] * (GIDX // 16)],
                           i16, kind='ExternalInput') for r in range(3)]
    st = [nc.dram_tensor(f'st{r}', [128, total_chunks[r] * 128],
                         bf16, kind='ExternalInput') for r in range(3)]
    sx = [nc.dram_tensor(f'sx{r}', [128, total_chunks[r] * 128],
                         bf16, kind='ExternalInput') for r in range(3)]
    hsk = nc.dram_tensor('hsk', [2 * QTR, D], fp32, kind='ExternalInput')
    outp = nc.dram_tensor('outp', [2 * QTR, D], fp32, kind='ExternalOutput')

    htab = [h0b, h1b]

    with tile.TileContext(nc) as tc:
        with (
            tc.tile_pool(name='singles', bufs=1) as singles,
            tc.tile_pool(name='qtmp', bufs=1) as qtmp,
            tc.tile_pool(name='gpool', bufs=3) as gpool,
            tc.tile_pool(name='spool', bufs=3) as spool,
            tc.tile_pool(name='work', bufs=6) as work,
            tc.tile_pool(name='opool', bufs=3) as opool,
            tc.tile_pool(name='psKV', bufs=2, space='PSUM') as psKV,
            tc.tile_pool(name='psQE', bufs=2, space='PSUM') as psQE,
            tc.tile_pool(name='psU', bufs=2, space='PSUM') as psU,
            tc.tile_pool(name='psT', bufs=1, space='PSUM') as psT,
            tc.tile_pool(name='psO', bufs=1, space='PSUM') as psO,
        ):
            from concourse import library_config
            nc.gpsimd.load_library(library_config.mlp)
            gidx_reg = nc.gpsimd.to_reg(GIDX)

            ident = singles.tile([128, 128], bf16)
            make_identity(nc, ident[:])

            # resident weights
            wq_sb = singles.tile([128, 2, 2, D], bf16)
            nc.sync.dma_start(out=wq_sb[:],
                              in_=wq[:].rearrange('a b p n -> p a b n'))
            wkv_sb = singles.tile([128, 3, 2, 2 * D], bf16)
            nc.sync.dma_start(out=wkv_sb[:],
                              in_=wkv[:].rearrange('a b p n -> p a b n'))
            wa_sb = singles.tile([128, 2, 2, D], bf16)
            nc.sync.dma_start(out=wa_sb[:],
                              in_=wa[:].rearrange('a b p n -> p a b n'))
            if use_bq:
                bq_sb = singles.tile([128, 2, D], fp32)
                for t in range(2):
                    nc.sync.dma_start(
                        out=bq_sb[:, t, :],
                        in_=bq2[t:t + 1, :].to_broadcast([128, D]))
            if use_bv:
                bv_sb = singles.tile([128, 2, D], fp32)
                for t in range(2):
                    nc.sync.dma_start(
                        out=bv_sb[:, t, :],
                        in_=bvt[t:t + 1, :].to_broadcast([128, D]))

            # resident gather indices
            sidx_sb = []
            for r in range(3):
                t1 = singles.tile([128, ngroups[r] * (GIDX // 16)], i16,
                                  tag=f'sidx{r}')
                nc.sync.dma_start(out=t1[:], in_=sidx[r][:])
                sidx_sb.append(t1)

            # Q tables (own dst rows) and rel0 t-store, resident in SBUF
            q_sb = []
            for t in range(2):
                qt = singles.tile([128, W, D], bf16, tag=f'q{t}')
                q_sb.append(qt)
            tstore = singles.tile([128, W, D], bf16)

            # ---------------- phase: build Q in SBUF ----------------
            hT = qtmp.tile([128, 2, 2, QTR], bf16)
            nc.sync.dma_start(out=hT[:],
                              in_=hownT[:].rearrange('a b p n -> p a b n'))
            for t in range(2):
                for w in range(W):
                    qp = psQE.tile([128, D], fp32, tag='qe')
                    for cc in range(2):
                        nc.tensor.matmul(
                            out=qp[:],
                            lhsT=hT[:, t, cc, w * 128:(w + 1) * 128],
                            rhs=wq_sb[:, t, cc, :],
                            start=(cc == 0), stop=(cc == 1))
                    if use_bq:
                        nc.vector.tensor_add(q_sb[t][:, w, :], qp[:],
                                             bq_sb[:, t, :])
                    else:
                        nc.scalar.activation(
                            q_sb[t][:, w, :], qp[:],
                            mybir.ActivationFunctionType.Copy)

            # ---------------- relation passes ----------------
            cw0r = [np.concatenate([[0], np.cumsum(chunks_w[r])])
                    for r in range(3)]

            def rel_pass(r, mode):
                # mode: 'out' (rel1), 'store' (rel0), 'combine' (rel2)
                side = DST_OF_REL[r]
                half = 0.5 if r != 1 else 1.0
                out_base = side * QTR
                cw = chunks_w[r]
                pending = []

                def flush(item):
                    w, u = item
                    if K_SKIP_FLUSH:
                        return
                    rcp = work.tile([128, H], fp32, tag='rcp')
                    nc.vector.tensor_scalar_add(rcp[:], u[:, D:D + H], 1e-20)
                    nc.vector.reciprocal(rcp[:], rcp[:])
                    if half != 1.0:
                        nc.vector.tensor_scalar_mul(rcp[:], rcp[:], half)
                    rcpb = rcp[:]
                    rcpb = bass.AP(tensor=rcpb.tensor, offset=rcpb.offset,
                                   ap=[*rcpb.ap, [0, DK]])
                    if mode == 'store':
                        nc.vector.tensor_mul(
                            tstore[:, w, :].rearrange('p (h d) -> p h d',
                                                      d=DK),
                            u[:, :D].rearrange('p (h d) -> p h d', d=DK),
                            rcpb)
                        return
                    t_sb = opool.tile([128, D], bf16, tag='t')
                    nc.vector.tensor_mul(
                        t_sb[:].rearrange('p (h d) -> p h d', d=DK),
                        u[:, :D].rearrange('p (h d) -> p h d', d=DK),
                        rcpb)
                    if mode == 'combine':
                        nc.vector.tensor_add(t_sb[:], t_sb[:],
                                             tstore[:, w, :])
                    if use_bv:
                        nc.vector.tensor_add(t_sb[:], t_sb[:],
                                             bv_sb[:, side, :])
                    tts = opool.tile([128, 2, 128], bf16, tag='tts')
                    for cc in range(2):
                        tp = psT.tile([128, 128], bf16, tag='tp')
                        nc.tensor.transpose(
                            tp[:], t_sb[:, cc * 128:(cc + 1) * 128],
                            ident[:])
                        nc.scalar.activation(
                            tts[:, cc, :], tp[:],
                            mybir.ActivationFunctionType.Copy)
                    om = psO.tile([128, D], fp32, tag='om')
                    for cc in range(2):
                        nc.tensor.matmul(
                            out=om[:], lhsT=tts[:, cc, :],
                            rhs=wa_sb[:, side, cc, :],
                            start=(cc == 0), stop=(cc == 1))
                    hs_t = opool.tile([128, D], fp32, tag='hs')
                    row = out_base + w * 128
                    nc.sync.dma_start(out=hs_t[:],
                                      in_=hsk[row:row + 128, :])
                    o_sb = opool.tile([128, D], fp32, tag='o')
                    nc.vector.scalar_tensor_tensor(
                        out=o_sb[:], in0=om[:], scalar=alpha[side],
                        in1=hs_t[:], op0=mybir.AluOpType.mult,
                        op1=mybir.AluOpType.add)
                    nc.sync.dma_start(out=outp[row:row + 128, :],
                                      in_=o_sb[:])

                ci = 0
                sg = None
                for w in range(W):
                    u = psU.tile([128, D + H], fp32, tag='u')
                    stt = spool.tile([128, cw[w] * 128], bf16, tag='st')
                    nc.sync.dma_start(
                        out=stt[:],
                        in_=st[r][:, cw0r[r][w] * 128:
                                  (cw0r[r][w] + cw[w]) * 128])
                    sxt = spool.tile([128, cw[w] * 128], bf16, tag='sx')
                    nc.sync.dma_start(
                        out=sxt[:],
                        in_=sx[r][:, cw0r[r][w] * 128:
                                  (cw0r[r][w] + cw[w]) * 128])
                    for j in range(cw[w]):
                        if ci % GCH == 0:
                            g = ci // GCH
                            sg = gpool.tile([128, 2, GIDX], bf16, tag='gs')
                            if not K_SKIP_GATHER:
                                nc.gpsimd.dma_gather(
                                    out_ap=sg[:],
                                    in_ap=htab[SRC_OF_REL[r]][:],
                                    idxs_ap=sidx_sb[r][
                                        :, g * (GIDX // 16):
                                        (g + 1) * (GIDX // 16)],
                                    num_idxs=GIDX, num_idxs_reg=gidx_reg,
                                    elem_size=D, transpose=True)
                            else:
                                nc.vector.memset(sg[:], 0.0)
                        col = (ci % GCH) * 128
                        kv = psKV.tile([128, 2 * D], fp32, tag='kv')
                        for cc in range(2):
                            nc.tensor.matmul(
                                out=kv[:],
                                lhsT=sg[:, cc, col:col + 128],
                                rhs=wkv_sb[:, r, cc, :],
                                start=(cc == 0), stop=(cc == 1))
                        qe = psQE.tile([128, D], fp32, tag='qe')
                        nc.tensor.matmul(
                            out=qe[:], lhsT=stt[:, j * 128:(j + 1) * 128],
                            rhs=q_sb[side][:, w, :], start=True, stop=True)
                        qe_bf = work.tile([128, D], bf16, tag='qeb')
                        nc.scalar.activation(
                            qe_bf[:], qe[:],
                            mybir.ActivationFunctionType.Copy)
                        p = work.tile([128, D], bf16, tag='p')
                        nc.vector.tensor_mul(p[:], kv[:, :D], qe_bf[:])
                        s8 = work.tile([128, H], fp32, tag='s8')
                        nc.vector.reduce_sum(
                            s8[:], p[:].rearrange('p (h d) -> p h d', d=DK),
                            axis=mybir.AxisListType.X)
                        rhs = work.tile([128, D + H], bf16, tag='rhs')
                        nc.scalar.activation(
                            rhs[:, D:D + H], s8[:],
                            mybir.ActivationFunctionType.Exp)
                        exb = rhs[:, D:D + H]
                        exb = bass.AP(tensor=exb.tensor, offset=exb.offset,
                                      ap=[*exb.ap, [0, DK]])
                        nc.vector.tensor_mul(
                            rhs[:, :D].rearrange('p (h d) -> p h d', d=DK),
                            kv[:, D:].rearrange('p (h d) -> p h d', d=DK),
                            exb)
                        nc.tensor.matmul(
                            out=u[:], lhsT=sxt[:, j * 128:(j + 1) * 128],
                            rhs=rhs[:], start=(j == 0), stop=(j == cw[w] - 1))
                        ci += 1
                        if j == 0 and pending:
                            flush(pending.pop(0))
                    pending.append((w, u))
                while pending:
                    flush(pending.pop(0))

            if not K_SKIP_REL:
                rel_pass(1, 'out')
                rel_pass(0, 'store')
                rel_pass(2, 'combine')
            if K_SKIP_REL or K_SKIP_FLUSH:
                nc.sync.dma_start(out=outp[:], in_=hsk[:])

    nc.compile()
    return nc


# ----------------------------------------------------------------------------
# Entry point
# ----------------------------------------------------------------------------

def _run(inputs, trace=False):
    in_maps, meta = prep(inputs)
    key = (meta['use_bq'], meta['use_bv'], meta['alpha'], meta['chunks_w'])
    if key not in _cache:
        _cache[key] = build_program(meta)
    nc = _cache[key]
    res = run_bass_kernel_spmd(nc, in_maps, core_ids=list(range(NCORES)),
                               trace=trace)
    out = np.zeros((2, N, D), np.float32)
    for c in range(NCORES):
        lo = c * RPC
        op = res.results[c]['outp']
        out[0, lo:lo + RPC] = op[:RPC]
        out[1, lo:lo + RPC] = op[QTR:QTR + RPC]
    return out, res


def kernel(**inputs):
    out, _ = _run(inputs, trace=False)
    return out
